# revision 4
# baseline (speedup 1.0000x reference)
"""MoE routing kernel (top-12-of-16 gating + dense expert FFN) for 8 Trainium2
NeuronCores.

Strategy: data-parallel shard of tokens (N=65536 -> 8192/core), weights
replicated. Per core, everything is computed feature-major (tokens on the
matmul free dim):
  - host pre-transposes x so no on-device transpose of x is needed
  - gating logits in fp32 (top-k selection is precision-critical)
  - both big FFN matmuls in bf16 with fp32 PSUM accumulation
  - gates broadcast across partitions via gpsimd.partition_broadcast
  - y is produced transposed [8, n_loc] and transposed back on host
  - per-core expert load counts are partial; host sums them
"""

import numpy as np
import ml_dtypes

import concourse.bacc as bacc
import concourse.tile as tile
import concourse.mybir as mybir
from concourse.bass_utils import run_bass_kernel_spmd
from concourse.masks import make_identity

F32 = mybir.dt.float32
BF16 = mybir.dt.bfloat16
AF = mybir.ActivationFunctionType
ALU = mybir.AluOpType

N_CORES = 8
N, D, E, H, OUT = 65536, 512, 16, 128, 8
TOPK = 12
N_LOC = N // N_CORES          # 8192
P = 128
DC = D // P                   # 4 contraction chunks of 128
T = 512                       # tokens per tile
NT = N_LOC // T               # 16 tiles per core
TS = T // P                   # 4 sub-tiles of 128 tokens

_BUILD_CACHE = {}


def _build(include_b2: bool):
    nc = bacc.Bacc(None, target_bir_lowering=False, debug=False)

    xt_d = nc.dram_tensor("xt", [P, DC, N_LOC], F32, kind="ExternalInput")
    w1_d = nc.dram_tensor("w1", [P, DC, E * H], BF16, kind="ExternalInput")
    w2_d = nc.dram_tensor("w2", [P, E, D], BF16, kind="ExternalInput")
    wg_d = nc.dram_tensor("wg", [P, DC, E], F32, kind="ExternalInput")
    wo_d = nc.dram_tensor("wo", [P, DC, OUT], BF16, kind="ExternalInput")
    b1_d = nc.dram_tensor("b1t", [P, E], F32, kind="ExternalInput")
    if include_b2:
        b2_d = nc.dram_tensor("b2", [E, DC, P], F32, kind="ExternalInput")

    yt_d = nc.dram_tensor("yt", [OUT, N_LOC], F32, kind="ExternalOutput")
    ga_d = nc.dram_tensor("gates", [N_LOC, E], F32, kind="ExternalOutput")
    ld_d = nc.dram_tensor("load_acc", [P, TS * E], F32, kind="ExternalOutput")

    with tile.TileContext(nc) as tc:
        with tc.tile_pool(name="const", bufs=1) as cpool, \
             tc.tile_pool(name="xin", bufs=3) as xpool, \
             tc.tile_pool(name="xb", bufs=2) as xbpool, \
             tc.tile_pool(name="gat", bufs=2) as gpool, \
             tc.tile_pool(name="hwp", bufs=2) as hwpool, \
             tc.tile_pool(name="sm", bufs=2) as smpool, \
             tc.tile_pool(name="h2p", bufs=2) as h2pool, \
             tc.tile_pool(name="plog", bufs=1, space="PSUM") as plog, \
             tc.tile_pool(name="pgt", bufs=1, space="PSUM") as pgt, \
             tc.tile_pool(name="ph", bufs=2, space="PSUM") as phpool, \
             tc.tile_pool(name="pout", bufs=4, space="PSUM") as popool:

            # --- constants / weights resident in SBUF ---
            w1_sb = cpool.tile([P, DC, E * H], BF16, tag="w1_sb")
            nc.sync.dma_start(w1_sb[:], w1_d[:])
            w2_sb = cpool.tile([P, E, D], BF16, tag="w2_sb")
            nc.sync.dma_start(w2_sb[:], w2_d[:])
            wg_sb = cpool.tile([P, DC, E], F32, tag="wg_sb")
            nc.sync.dma_start(wg_sb[:], wg_d[:])
            wo_sb = cpool.tile([P, DC, OUT], BF16, tag="wo_sb")
            nc.sync.dma_start(wo_sb[:], wo_d[:])
            b1_sb = cpool.tile([P, E], F32, tag="b1_sb")
            nc.sync.dma_start(b1_sb[:], b1_d[:])
            if include_b2:
                b2_sb = cpool.tile([E, DC, P], F32, tag="b2_sb")
                nc.sync.dma_start(b2_sb[:], b2_d[:])
            ident = cpool.tile([P, P], F32, tag="ident")
            make_identity(nc, ident[:])
            acc_sb = cpool.tile([P, TS, E], F32, tag="acc_sb")
            nc.vector.memset(acc_sb[:], 0.0)

            for i in range(NT):
                t0 = i * T
                # --- load x tile (feature-major, pre-transposed on host) ---
                xtf = xpool.tile([P, DC, T], F32, tag="xtf")
                nc.sync.dma_start(xtf[:], xt_d[:, :, t0:t0 + T])
                xtb = xbpool.tile([P, DC, T], BF16, tag="xtb")
                nc.vector.tensor_copy(xtb[:], xtf[:])

                # --- gating logits (fp32): psum[128t, E] per sub-tile ---
                lp = plog.tile([P, TS * E], F32, tag="lp")
                for s in range(TS):
                    for d in range(DC):
                        nc.tensor.matmul(
                            lp[:, s * E:(s + 1) * E],
                            lhsT=xtf[:, d, s * P:(s + 1) * P],
                            rhs=wg_sb[:, d, :],
                            start=(d == 0), stop=(d == DC - 1),
                        )
                l_sb = smpool.tile([P, TS, E], F32, tag="l_sb")
                nc.vector.tensor_copy(l_sb[:], lp[:].rearrange("p (s e) -> p s e", e=E))

                # --- top-12-of-16 mask + softmax ---
                m1 = smpool.tile([P, TS, 8], F32, tag="m1")
                m2 = smpool.tile([P, TS, 8], F32, tag="m2")
                wrk = smpool.tile([P, TS, E], F32, tag="wrk")
                for s in range(TS):
                    nc.vector.max(m1[:, s, :], l_sb[:, s, :])
                    nc.vector.match_replace(
                        out=wrk[:, s, :], in_to_replace=m1[:, s, :],
                        in_values=l_sb[:, s, :], imm_value=-1e30,
                    )
                    nc.vector.max(m2[:, s, :], wrk[:, s, :])
                # tau = 12th largest = m2[:,:,3]; rowmax = m1[:,:,0]
                mask = smpool.tile([P, TS, E], F32, tag="mask")
                nc.vector.tensor_tensor(
                    mask[:], l_sb[:],
                    m2[:, :, 3:4].broadcast_to([P, TS, E]), ALU.is_ge)
                dsh = smpool.tile([P, TS, E], F32, tag="dsh")
                nc.vector.tensor_tensor(
                    dsh[:], l_sb[:],
                    m1[:, :, 0:1].broadcast_to([P, TS, E]), ALU.subtract)
                ex = smpool.tile([P, TS, E], F32, tag="ex")
                nc.scalar.activation(ex[:], dsh[:], AF.Exp)
                nc.vector.tensor_tensor(ex[:], ex[:], mask[:], ALU.mult)
                zs = smpool.tile([P, TS], F32, tag="zs")
                nc.vector.tensor_reduce(zs[:], ex[:], mybir.AxisListType.X, ALU.add)
                rz = smpool.tile([P, TS], F32, tag="rz")
                nc.vector.reciprocal(rz[:], zs[:])
                gts = gpool.tile([P, TS, E], F32, tag="gts")
                nc.vector.tensor_tensor(
                    gts[:], ex[:],
                    rz[:, :, None].broadcast_to([P, TS, E]), ALU.mult)
                nc.vector.tensor_tensor(acc_sb[:], acc_sb[:], mask[:], ALU.add)
                nc.sync.dma_start(
                    ga_d[t0:t0 + T, :].rearrange("(s p) e -> p s e", p=P), gts[:])

                # --- transpose gates -> gT [E, T] (fp32 + bf16 copies) ---
                gt_sb = gpool.tile([E, T], F32, tag="gt_sb")
                for s in range(TS):
                    gt_ps = pgt.tile([E, P], F32, tag="gt_ps")
                    nc.tensor.transpose(gt_ps[:], gts[:, s, :], ident[:])
                    nc.vector.tensor_copy(gt_sb[:, s * P:(s + 1) * P], gt_ps[:])
                gtb = gpool.tile([E, T], BF16, tag="gtb")
                nc.vector.tensor_copy(gtb[:], gt_sb[:])
                # flatten all 16 gate rows onto partition 0 so
                # gpsimd.partition_broadcast (base-partition-0 only) can use it
                gtf = gpool.tile([1, E, T], BF16, tag="gtf")
                nc.sync.dma_start(gtf[:], gtb[:])

                # --- expert FFN part 1: h = relu(x@W1+b1), hw = h*g ---
                hw = hwpool.tile([P, E, T], BF16, tag="hw")
                for e in range(E):
                    ph = phpool.tile([P, T], F32, tag="ph")
                    for d in range(DC):
                        nc.tensor.matmul(
                            ph[:],
                            lhsT=w1_sb[:, d, e * H:(e + 1) * H],
                            rhs=xtb[:, d, :],
                            start=(d == 0), stop=(d == DC - 1),
                        )
                    hr = hwpool.tile([P, T], BF16, tag="hr")
                    nc.scalar.activation(hr[:], ph[:], AF.Relu,
                                         bias=b1_sb[:, e:e + 1])
                    ge = hwpool.tile([P, T], BF16, tag="ge")
                    nc.gpsimd.partition_broadcast(ge[:], gtf[:, e, :])
                    nc.vector.tensor_tensor(hw[:, e, :], hr[:], ge[:], ALU.mult)

                # --- expert FFN part 2: out2 = hw@W2 (+gates@b2), resid, y ---
                h2 = h2pool.tile([P, DC, T], BF16, tag="h2")
                for d in range(DC):
                    po = popool.tile([P, T], F32, tag="pout")
                    if include_b2:
                        nc.tensor.matmul(
                            po[:], lhsT=b2_sb[:, d, :], rhs=gt_sb[:],
                            start=True, stop=False)
                    for e in range(E):
                        nc.tensor.matmul(
                            po[:],
                            lhsT=w2_sb[:, e, d * P:(d + 1) * P],
                            rhs=hw[:, e, :],
                            start=(e == 0 and not include_b2),
                            stop=(e == E - 1),
                        )
                    h2a = h2pool.tile([P, T], F32, tag="h2a")
                    nc.scalar.activation(h2a[:], po[:], AF.Relu)
                    nc.vector.tensor_tensor(h2[:, d, :], h2a[:], xtf[:, d, :],
                                            ALU.add)

                py = popool.tile([OUT, T], F32, tag="pout")
                for d in range(DC):
                    nc.tensor.matmul(
                        py[:], lhsT=wo_sb[:, d, :], rhs=h2[:, d, :],
                        start=(d == 0), stop=(d == DC - 1),
                    )
                y_sb = gpool.tile([OUT, T], F32, tag="y_sb")
                nc.vector.tensor_copy(y_sb[:], py[:])
                nc.sync.dma_start(yt_d[:, t0:t0 + T], y_sb[:])

            nc.sync.dma_start(ld_d[:], acc_sb[:].rearrange("p s e -> p (s e)"))

    nc.finalize()
    return nc


def _get_nc(include_b2: bool):
    key = (include_b2,)
    if key not in _BUILD_CACHE:
        _BUILD_CACHE[key] = _build(include_b2)
    return _BUILD_CACHE[key]


def kernel(x, modality, Wg, W1, b1, W2, b2, Wout, bout):
    x = np.asarray(x, dtype=np.float32)
    Wg = np.asarray(Wg, dtype=np.float32)
    W1 = np.asarray(W1, dtype=np.float32)
    b1 = np.asarray(b1, dtype=np.float32)
    W2 = np.asarray(W2, dtype=np.float32)
    b2 = np.asarray(b2, dtype=np.float32)
    Wout = np.asarray(Wout, dtype=np.float32)
    bout = np.asarray(bout, dtype=np.float32)
    mod = int(np.asarray(modality))

    assert x.shape == (N, D)
    include_b2 = bool(np.any(b2))
    nc = _get_nc(include_b2)

    # ---- host-side prep into device layouts ----
    bf = ml_dtypes.bfloat16
    # W1f[d, e*H+h] = W1[e, d, h] -> [P, DC, E*H]
    w1f = np.ascontiguousarray(
        W1.transpose(1, 0, 2).reshape(D, E * H).reshape(DC, P, E * H)
        .transpose(1, 0, 2)).astype(bf)
    # W2f[(e,h), d] = W2[e, h, d] -> [P(h), E, D]
    w2f = np.ascontiguousarray(W2.transpose(1, 0, 2)).astype(bf)
    wgm = np.ascontiguousarray(
        Wg[mod].reshape(DC, P, E).transpose(1, 0, 2)).astype(np.float32)
    wof = np.ascontiguousarray(
        Wout.reshape(DC, P, OUT).transpose(1, 0, 2)).astype(bf)
    b1t = np.ascontiguousarray(b1.T).astype(np.float32)          # [P, E]
    b2f = np.ascontiguousarray(b2.reshape(E, DC, P)).astype(np.float32)

    base = {
        "w1": w1f, "w2": w2f, "wg": wgm, "wo": wof, "b1t": b1t,
    }
    if include_b2:
        base["b2"] = b2f

    in_maps = []
    for c in range(N_CORES):
        xs = x[c * N_LOC:(c + 1) * N_LOC]                        # [N_LOC, D]
        xtf = np.ascontiguousarray(
            xs.T.reshape(DC, P, N_LOC).transpose(1, 0, 2)).astype(np.float32)
        m = dict(base)
        m["xt"] = xtf
        in_maps.append(m)

    res = run_bass_kernel_spmd(nc, in_maps, core_ids=list(range(N_CORES)))

    y = np.empty((N, OUT), dtype=np.float32)
    gates = np.empty((N, E), dtype=np.float32)
    load = np.zeros((E,), dtype=np.float32)
    for c, r in enumerate(res.results):
        y[c * N_LOC:(c + 1) * N_LOC] = r["yt"].T
        gates[c * N_LOC:(c + 1) * N_LOC] = r["gates"]
        load += r["load_acc"].reshape(P, TS, E).sum(axis=(0, 1))
    if np.any(bout):
        y += bout[None, :]
    return (y, gates, load)


# revision 6
# speedup vs baseline: 1.3070x; 1.3070x over previous
"""MoE routing kernel (top-12-of-16 gating + dense expert FFN) for 8 Trainium2
NeuronCores.

Strategy: data-parallel shard of tokens (N=65536 -> 8192/core), weights
replicated. Per core, everything is computed feature-major (tokens on the
matmul free dim):
  - host pre-transposes x so no on-device transpose of x is needed
  - gating logits in fp32 (top-k selection is precision-critical)
  - both big FFN matmuls in bf16 with fp32 PSUM accumulation
  - gates broadcast across partitions via gpsimd.partition_broadcast
  - y is produced transposed [8, n_loc] and transposed back on host
  - per-core expert load counts are partial; host sums them
"""

import numpy as np
import ml_dtypes

import concourse.bacc as bacc
import concourse.tile as tile
import concourse.mybir as mybir
from concourse.bass_utils import run_bass_kernel_spmd
from concourse.masks import make_identity

F32 = mybir.dt.float32
BF16 = mybir.dt.bfloat16
AF = mybir.ActivationFunctionType
ALU = mybir.AluOpType

N_CORES = 8
N, D, E, H, OUT = 65536, 512, 16, 128, 8
TOPK = 12
N_LOC = N // N_CORES          # 8192
P = 128
DC = D // P                   # 4 contraction chunks of 128
T = 512                       # tokens per tile
NT = N_LOC // T               # 16 tiles per core
TS = T // P                   # 4 sub-tiles of 128 tokens

_BUILD_CACHE = {}


def _build(include_b2: bool):
    nc = bacc.Bacc(None, target_bir_lowering=False, debug=False)

    xt_d = nc.dram_tensor("xt", [P, DC, N_LOC], F32, kind="ExternalInput")
    w1_d = nc.dram_tensor("w1", [P, DC, E * H], BF16, kind="ExternalInput")
    w2_d = nc.dram_tensor("w2", [P, E, D], BF16, kind="ExternalInput")
    wg_d = nc.dram_tensor("wg", [P, DC, E], F32, kind="ExternalInput")
    wo_d = nc.dram_tensor("wo", [P, DC, OUT], BF16, kind="ExternalInput")
    b1_d = nc.dram_tensor("b1t", [P, E], F32, kind="ExternalInput")
    if include_b2:
        b2_d = nc.dram_tensor("b2", [E, DC, P], F32, kind="ExternalInput")

    yt_d = nc.dram_tensor("yt", [OUT, N_LOC], F32, kind="ExternalOutput")
    ga_d = nc.dram_tensor("gates", [N_LOC, E], F32, kind="ExternalOutput")
    ld_d = nc.dram_tensor("load_acc", [P, TS * E], F32, kind="ExternalOutput")

    with tile.TileContext(nc) as tc:
        with tc.tile_pool(name="const", bufs=1) as cpool, \
             tc.tile_pool(name="xin", bufs=4) as xpool, \
             tc.tile_pool(name="xb", bufs=3) as xbpool, \
             tc.tile_pool(name="gat", bufs=3) as gpool, \
             tc.tile_pool(name="hwp", bufs=2) as hwpool, \
             tc.tile_pool(name="hrp", bufs=4) as hrpool, \
             tc.tile_pool(name="gep", bufs=8) as gepool, \
             tc.tile_pool(name="gdr", bufs=2, space="DRAM") as gdram, \
             tc.tile_pool(name="sm", bufs=2) as smpool, \
             tc.tile_pool(name="h2p", bufs=3) as h2pool, \
             tc.tile_pool(name="plog", bufs=1, space="PSUM") as plog, \
             tc.tile_pool(name="pgt", bufs=1, space="PSUM") as pgt, \
             tc.tile_pool(name="ph", bufs=2, space="PSUM") as phpool, \
             tc.tile_pool(name="pout", bufs=4, space="PSUM") as popool:

            # --- constants / weights resident in SBUF ---
            # (small gating consts on the sync queue, bulky weights on the
            # scalar queue so tile 0's x/wg loads aren't stuck behind them)
            wg_sb = cpool.tile([P, DC, E], F32, tag="wg_sb")
            nc.sync.dma_start(wg_sb[:], wg_d[:])
            b1_sb = cpool.tile([P, E], F32, tag="b1_sb")
            nc.sync.dma_start(b1_sb[:], b1_d[:])
            w1_sb = cpool.tile([P, DC, E * H], BF16, tag="w1_sb")
            nc.scalar.dma_start(w1_sb[:], w1_d[:])
            w2_sb = cpool.tile([P, E, D], BF16, tag="w2_sb")
            nc.scalar.dma_start(w2_sb[:], w2_d[:])
            wo_sb = cpool.tile([P, DC, OUT], BF16, tag="wo_sb")
            nc.scalar.dma_start(wo_sb[:], wo_d[:])
            if include_b2:
                b2_sb = cpool.tile([E, DC, P], F32, tag="b2_sb")
                nc.scalar.dma_start(b2_sb[:], b2_d[:])
            ident = cpool.tile([P, P], F32, tag="ident")
            make_identity(nc, ident[:])
            acc_sb = cpool.tile([P, TS, E], F32, tag="acc_sb")
            nc.vector.memset(acc_sb[:], 0.0)

            for i in range(NT):
                t0 = i * T
                # --- load x tile (feature-major, pre-transposed on host) ---
                xtf = xpool.tile([P, DC, T], F32, tag="xtf")
                nc.sync.dma_start(xtf[:], xt_d[:, :, t0:t0 + T])
                xtb = xbpool.tile([P, DC, T], BF16, tag="xtb")
                nc.vector.tensor_copy(xtb[:], xtf[:])

                # --- gating logits (fp32): psum[128t, E] per sub-tile ---
                lp = plog.tile([P, TS * E], F32, tag="lp")
                for s in range(TS):
                    for d in range(DC):
                        nc.tensor.matmul(
                            lp[:, s * E:(s + 1) * E],
                            lhsT=xtf[:, d, s * P:(s + 1) * P],
                            rhs=wg_sb[:, d, :],
                            start=(d == 0), stop=(d == DC - 1),
                        )
                l_sb = smpool.tile([P, TS, E], F32, tag="l_sb")
                nc.vector.tensor_copy(l_sb[:], lp[:].rearrange("p (s e) -> p s e", e=E))

                # --- top-12-of-16 mask + softmax ---
                m1 = smpool.tile([P, TS, 8], F32, tag="m1")
                m2 = smpool.tile([P, TS, 8], F32, tag="m2")
                wrk = smpool.tile([P, TS, E], F32, tag="wrk")
                for s in range(TS):
                    nc.vector.max(m1[:, s, :], l_sb[:, s, :])
                    nc.vector.match_replace(
                        out=wrk[:, s, :], in_to_replace=m1[:, s, :],
                        in_values=l_sb[:, s, :], imm_value=-1e30,
                    )
                    nc.vector.max(m2[:, s, :], wrk[:, s, :])
                # tau = 12th largest = m2[:,:,3]; rowmax = m1[:,:,0]
                mask = smpool.tile([P, TS, E], F32, tag="mask")
                nc.vector.tensor_tensor(
                    mask[:], l_sb[:],
                    m2[:, :, 3:4].broadcast_to([P, TS, E]), ALU.is_ge)
                dsh = smpool.tile([P, TS, E], F32, tag="dsh")
                nc.vector.tensor_tensor(
                    dsh[:], l_sb[:],
                    m1[:, :, 0:1].broadcast_to([P, TS, E]), ALU.subtract)
                ex = smpool.tile([P, TS, E], F32, tag="ex")
                nc.scalar.activation(ex[:], dsh[:], AF.Exp)
                nc.vector.tensor_tensor(ex[:], ex[:], mask[:], ALU.mult)
                zs = smpool.tile([P, TS], F32, tag="zs")
                nc.vector.tensor_reduce(zs[:], ex[:], mybir.AxisListType.X, ALU.add)
                rz = smpool.tile([P, TS], F32, tag="rz")
                nc.vector.reciprocal(rz[:], zs[:])
                gts = gpool.tile([P, TS, E], F32, tag="gts")
                nc.vector.tensor_tensor(
                    gts[:], ex[:],
                    rz[:, :, None].broadcast_to([P, TS, E]), ALU.mult)
                nc.vector.tensor_tensor(acc_sb[:], acc_sb[:], mask[:], ALU.add)
                nc.sync.dma_start(
                    ga_d[t0:t0 + T, :].rearrange("(s p) e -> p s e", p=P), gts[:])

                # --- expert FFN part 1: h = relu(x@W1+b1), hw = h*g ---
                # Gates transpose / broadcast chain is emitted after expert 0's
                # matmuls so the PE doesn't stall on the DVE softmax chain.
                hw = hwpool.tile([P, E, T], BF16, tag="hw")
                hrs = []
                for e in range(2):
                    ph = phpool.tile([P, T], F32, tag="ph")
                    for d in range(DC):
                        nc.tensor.matmul(
                            ph[:],
                            lhsT=w1_sb[:, d, e * H:(e + 1) * H],
                            rhs=xtb[:, d, :],
                            start=(d == 0), stop=(d == DC - 1),
                        )
                    hr = hrpool.tile([P, T], BF16, tag="hr")
                    nc.scalar.activation(hr[:], ph[:], AF.Relu,
                                         bias=b1_sb[:, e:e + 1])
                    hrs.append(hr)

                # --- transpose gates -> gT [E, T], bounce to DRAM, then
                # per-expert partition-broadcast via DMA reads ---
                gt_sb = gpool.tile([E, T], F32, tag="gt_sb")
                for s in range(TS):
                    gt_ps = pgt.tile([E, P], F32, tag="gt_ps")
                    nc.tensor.transpose(gt_ps[:], gts[:, s, :], ident[:])
                    nc.vector.tensor_copy(gt_sb[:, s * P:(s + 1) * P], gt_ps[:])
                gtb = gpool.tile([E, T], BF16, tag="gtb")
                nc.vector.tensor_copy(gtb[:], gt_sb[:])
                gtd = gdram.tile([E, T], BF16, tag="gtd")
                nc.sync.dma_start(gtd[:], gtb[:])
                ges = []
                for e in range(E):
                    ge = gepool.tile([P, T], BF16, tag="ge")
                    eng = nc.sync if e % 2 == 0 else nc.scalar
                    eng.dma_start(ge[:], gtd[e:e + 1, :].to_broadcast((P, T)))
                    ges.append(ge)

                for e in range(2):
                    nc.vector.tensor_tensor(hw[:, e, :], hrs[e][:], ges[e][:],
                                            ALU.mult)
                for e in range(2, E):
                    ph = phpool.tile([P, T], F32, tag="ph")
                    for d in range(DC):
                        nc.tensor.matmul(
                            ph[:],
                            lhsT=w1_sb[:, d, e * H:(e + 1) * H],
                            rhs=xtb[:, d, :],
                            start=(d == 0), stop=(d == DC - 1),
                        )
                    hr = hrpool.tile([P, T], BF16, tag="hr")
                    nc.scalar.activation(hr[:], ph[:], AF.Relu,
                                         bias=b1_sb[:, e:e + 1])
                    nc.vector.tensor_tensor(hw[:, e, :], hr[:], ges[e][:],
                                            ALU.mult)

                # --- expert FFN part 2: out2 = hw@W2 (+gates@b2), resid, y ---
                h2 = h2pool.tile([P, DC, T], BF16, tag="h2")
                for d in range(DC):
                    po = popool.tile([P, T], F32, tag="pout")
                    if include_b2:
                        nc.tensor.matmul(
                            po[:], lhsT=b2_sb[:, d, :], rhs=gt_sb[:],
                            start=True, stop=False)
                    for e in range(E):
                        nc.tensor.matmul(
                            po[:],
                            lhsT=w2_sb[:, e, d * P:(d + 1) * P],
                            rhs=hw[:, e, :],
                            start=(e == 0 and not include_b2),
                            stop=(e == E - 1),
                        )
                    h2a = h2pool.tile([P, T], F32, tag="h2a")
                    nc.scalar.activation(h2a[:], po[:], AF.Relu)
                    nc.vector.tensor_tensor(h2[:, d, :], h2a[:], xtf[:, d, :],
                                            ALU.add)

                py = popool.tile([OUT, T], F32, tag="pout")
                for d in range(DC):
                    nc.tensor.matmul(
                        py[:], lhsT=wo_sb[:, d, :], rhs=h2[:, d, :],
                        start=(d == 0), stop=(d == DC - 1),
                    )
                y_sb = gpool.tile([OUT, T], F32, tag="y_sb")
                nc.vector.tensor_copy(y_sb[:], py[:])
                nc.sync.dma_start(yt_d[:, t0:t0 + T], y_sb[:])

            nc.sync.dma_start(ld_d[:], acc_sb[:].rearrange("p s e -> p (s e)"))

    nc.finalize()
    return nc


def _get_nc(include_b2: bool):
    key = (include_b2,)
    if key not in _BUILD_CACHE:
        _BUILD_CACHE[key] = _build(include_b2)
    return _BUILD_CACHE[key]


def kernel(x, modality, Wg, W1, b1, W2, b2, Wout, bout):
    x = np.asarray(x, dtype=np.float32)
    Wg = np.asarray(Wg, dtype=np.float32)
    W1 = np.asarray(W1, dtype=np.float32)
    b1 = np.asarray(b1, dtype=np.float32)
    W2 = np.asarray(W2, dtype=np.float32)
    b2 = np.asarray(b2, dtype=np.float32)
    Wout = np.asarray(Wout, dtype=np.float32)
    bout = np.asarray(bout, dtype=np.float32)
    mod = int(np.asarray(modality))

    assert x.shape == (N, D)
    include_b2 = bool(np.any(b2))
    nc = _get_nc(include_b2)

    # ---- host-side prep into device layouts ----
    bf = ml_dtypes.bfloat16
    # W1f[d, e*H+h] = W1[e, d, h] -> [P, DC, E*H]
    w1f = np.ascontiguousarray(
        W1.transpose(1, 0, 2).reshape(D, E * H).reshape(DC, P, E * H)
        .transpose(1, 0, 2)).astype(bf)
    # W2f[(e,h), d] = W2[e, h, d] -> [P(h), E, D]
    w2f = np.ascontiguousarray(W2.transpose(1, 0, 2)).astype(bf)
    wgm = np.ascontiguousarray(
        Wg[mod].reshape(DC, P, E).transpose(1, 0, 2)).astype(np.float32)
    wof = np.ascontiguousarray(
        Wout.reshape(DC, P, OUT).transpose(1, 0, 2)).astype(bf)
    b1t = np.ascontiguousarray(b1.T).astype(np.float32)          # [P, E]
    b2f = np.ascontiguousarray(b2.reshape(E, DC, P)).astype(np.float32)

    base = {
        "w1": w1f, "w2": w2f, "wg": wgm, "wo": wof, "b1t": b1t,
    }
    if include_b2:
        base["b2"] = b2f

    in_maps = []
    for c in range(N_CORES):
        xs = x[c * N_LOC:(c + 1) * N_LOC]                        # [N_LOC, D]
        xtf = np.ascontiguousarray(
            xs.T.reshape(DC, P, N_LOC).transpose(1, 0, 2)).astype(np.float32)
        m = dict(base)
        m["xt"] = xtf
        in_maps.append(m)

    res = run_bass_kernel_spmd(nc, in_maps, core_ids=list(range(N_CORES)))

    y = np.empty((N, OUT), dtype=np.float32)
    gates = np.empty((N, E), dtype=np.float32)
    load = np.zeros((E,), dtype=np.float32)
    for c, r in enumerate(res.results):
        y[c * N_LOC:(c + 1) * N_LOC] = r["yt"].T
        gates[c * N_LOC:(c + 1) * N_LOC] = r["gates"]
        load += r["load_acc"].reshape(P, TS, E).sum(axis=(0, 1))
    if np.any(bout):
        y += bout[None, :]
    return (y, gates, load)
